# revision 37
# baseline (speedup 1.0000x reference)
"""Per-pixel adaptive 5x5 conv (KPN apply) on 8 Trainium2 NeuronCores.

out[b,c,h,w] = sum_{i,j} core[b,0,i*5+j,c,h,w] * frames[b,0,c,h+i-2,w+j-2]
(zero-padded borders), output [4,3,512,512] f32.

Sharding: pure data parallel, core k -> (b = k//2, H-half = k%2).

All device inputs are bfloat16 (truncated f32 high halves, taken as
zero-copy numpy views on the host so the only host-side gather happens
once, inside the runner). Halves every byte moved: host memcpy, host->
device transfer, and HBM traffic on-core. Kernel math: products and the
pairwise reduction tree in bf16 on DVE (2x packed mode), final add
emits f32. Measured on HW: rel err 7.4e-3, absmax ratio 9.9e-3 (gate
2e-2). TimelineSim per-core device estimate 104 us (baseline 203 us).

Raw-bass implementation (the walrus build in this env only allows one
semaphore wait per compute/DMA instruction, so Tile's auto-sync can't be
used): explicit double-buffered pipeline, standalone waits, all loads/
stores on the SP HWDGE FIFO so ordering among DMAs is implicit.

Per 128-row block: five DMAs bring the 25 tap planes [128, 5, 512] bf16
each (chunked so DVE can start after the first chunk lands), one DMA
brings a 5-row overlapping window of the padded frame. DVE computes the
products in five fused ops (one per kernel row i: out/in0 are the five
contiguous tap slices, in1 is an overlapping AP [(1,5),(1,512)] whose
five segments are the five column shifts of that frame row — measured
on HW via a Fori microbench at 1404 ns/op = 281 ns/tap, the full 2x
packed rate; segment misalignment costs nothing), then reduces with a
6-op pairwise tree over contiguous multi-tap slices; the last add
writes f32.
"""

import numpy as np
import ml_dtypes

import concourse.bass as bass
import concourse.mybir as mybir
from concourse.ap import AP
from concourse.bass_utils import run_bass_kernel_spmd

B, N, C, H, W = 4, 1, 3, 512, 512
K = 5
PAD = K // 2
NCORES = 8
HH = H // (NCORES // B)  # 256 rows per core
P = 128
NBLK_TOT = C * (HH // P)  # 6 blocks of 128 rows per core
WPAD = W + 2 * PAD        # 516
BF16 = ml_dtypes.bfloat16

_CACHE = {}


def _build():
    nc = bass.Bass()
    f32 = mybir.dt.float32
    bf16 = mybir.dt.bfloat16

    fr = nc.declare_dram_parameter("fr", [C, HH + 2 * PAD, WPAD], bf16, isOutput=False)
    co = nc.declare_dram_parameter("co", [K * K, C, HH, W], bf16, isOutput=False)
    out = nc.declare_dram_parameter("out", [C, HH, W], f32, isOutput=True)

    def co_view(n, g):
        # tap chunk g (5 taps) of block n, as [P, 5, W]
        c, blk = n // (HH // P), n % (HH // P)
        return co[5 * g:5 * g + 5, c, blk * P:blk * P + P, :].transpose([1, 0, 2])

    def fr_win(n):
        # [P, K, 516] window of the padded frame, rows overlapping
        c, blk = n // (HH // P), n % (HH // P)
        fb = fr[c, blk * P:blk * P + P, :]
        return AP(fb.tensor, fb.offset, [(WPAD, P), (WPAD, K), (1, WPAD)])

    def out_view(n):
        c, blk = n // (HH // P), n % (HH // P)
        return out[c, blk * P:blk * P + P, :]

    from contextlib import ExitStack
    with ExitStack() as ctx:
        e = ctx.enter_context
        # triple-buffered loads: the sim shows DMA keeping ~2.7us/block
        # ahead of DVE, but on hardware all 8 cores contend for HBM, so a
        # third buffer absorbs latency variance the sim does not model
        cts = [e(nc.sbuf_tensor(f"ct{i}", [P, K * K, W], bf16)) for i in range(3)]
        fas = [e(nc.sbuf_tensor(f"fa{i}", [P, K, WPAD], bf16)) for i in range(3)]
        prd = e(nc.sbuf_tensor("prd", [P, K * K, W], bf16))
        accs = [e(nc.sbuf_tensor(f"ac{i}", [P, W], bf16)) for i in range(2)]
        oas = [e(nc.sbuf_tensor(f"oa{i}", [P, W], f32)) for i in range(3)]
        # One semaphore per ct tap-chunk family: chunk g of every block
        # increments gsem[g], so gsem[g] >= 16*(n+1) needs all 16 engines
        # to have delivered block n's chunk-g increment, and per-engine
        # FIFO order of the SP HWDGE ring then guarantees every earlier
        # DMA (the frame window and chunks < g of block n) has drained too.
        # (A single cumulative count across different DMAs would race: a
        # fast engine can deliver a later DMA's increment while a slow
        # engine still owes one from an earlier DMA.)
        gsems = [e(nc.semaphore(f"gs{i}")) for i in range(K)]
        osem = e(nc.semaphore("osem"))  # store completions (+16 per DMA)
        wsem = e(nc.semaphore("wsem"))  # DVE per-block bf16 sum ready (+1)
        vsem = e(nc.semaphore("vsem"))  # ACT per-block f32 out ready (+1)
        xsem = e(nc.semaphore("xsem"))  # bookkeeping only (never waited on;
                                        # this walrus build requires sync
                                        # info on every dynamic DMA)
        NG = K  # 5 tap chunks per block
        NB = 3  # load/store buffer depth
        block = e(nc.Block())

        @block.sync
        def _(sync: bass.BassEngine):
            for n in range(NBLK_TOT):
                if n >= NB:
                    # DVE done with block n-NB => its buffers reusable
                    sync.wait_ge(vsem, n - NB + 1)
                sync.dma_start(out=fas[n % NB][:], in_=fr_win(n)).then_inc(xsem, 16)
                for g in range(NG):
                    sync.dma_start(
                        out=cts[n % NB][:, 5 * g:5 * g + 5, :], in_=co_view(n, g)
                    ).then_inc(gsems[g], 16)
                if n >= 2:
                    # DVE done with block n-2 => its f32 acc ready to store
                    sync.wait_ge(vsem, n - 1)
                    sync.dma_start(
                        out=out_view(n - 2), in_=oas[(n - 2) % NB][:]
                    ).then_inc(osem, 16)
            for m in (NBLK_TOT - 2, NBLK_TOT - 1):
                sync.wait_ge(vsem, m + 1)
                sync.dma_start(
                    out=out_view(m), in_=oas[m % NB][:]
                ).then_inc(osem, 16)
            sync.wait_ge(osem, 16 * NBLK_TOT)

        @block.vector
        def _(vector: bass.BassEngine):
            for n in range(NBLK_TOT):
                ct, fta = cts[n % NB], fas[n % NB]
                for i in range(K):
                    # this tap chunk (and everything before it) landed
                    vector.wait_ge(gsems[i], 16 * (n + 1))
                    # one fused op per kernel row i: the five segments of
                    # the overlapping in1 AP are the five column shifts
                    # j=0..4 of frame row i, paired with tap slices 5i+j
                    fbase = fta[:]
                    vector.tensor_mul(
                        out=prd[:, 5 * i:5 * i + 5, :],
                        in0=ct[:, 5 * i:5 * i + 5, :],
                        in1=AP(fbase.tensor, fbase.offset + i * WPAD,
                               [(K * WPAD, P), (1, K), (1, W)]))
                # pairwise tree over contiguous tap slices: 25 = (8+8+8)+1
                vector.tensor_add(
                    out=prd[:, 0:8, :], in0=prd[:, 0:8, :], in1=prd[:, 8:16, :])
                vector.tensor_add(
                    out=prd[:, 0:8, :], in0=prd[:, 0:8, :], in1=prd[:, 16:24, :])
                vector.tensor_add(
                    out=prd[:, 0:4, :], in0=prd[:, 0:4, :], in1=prd[:, 4:8, :])
                vector.tensor_add(
                    out=prd[:, 0:2, :], in0=prd[:, 0:2, :], in1=prd[:, 2:4, :])
                vector.tensor_add(
                    out=prd[:, 0, :], in0=prd[:, 0, :], in1=prd[:, 1, :])
                if n >= 2:
                    # ACT(n-2) drained this bf16 sum buffer
                    vector.wait_ge(vsem, n - 1)
                # final add stays pure-bf16 (2x packed); the idle scalar
                # engine does the f32 upconvert concurrently
                vector.tensor_add(
                    out=accs[n % 2][:], in0=prd[:, 0, :], in1=prd[:, 24, :]
                ).then_inc(wsem, 1)

        @block.scalar
        def _(scalar: bass.BassEngine):
            for n in range(NBLK_TOT):
                scalar.wait_ge(wsem, n + 1)
                if n >= NB:
                    # store of block n-NB (same f32 out buffer) must be done
                    scalar.wait_ge(osem, 16 * (n - NB + 1))
                scalar.activation(
                    out=oas[n % NB][:], in_=accs[n % 2][:],
                    func=mybir.ActivationFunctionType.Copy,
                ).then_inc(vsem, 1)
    return nc


def get_nc():
    if "nc" not in _CACHE:
        _CACHE["nc"] = _build()
    return _CACHE["nc"]


def _as_bf16_trunc(a):
    # Zero-copy bf16 view: the high 16 bits of each f32 (little-endian).
    # Truncation (not round-to-nearest); max rel err 2^-8 per element.
    return a.view(np.uint16)[..., 1::2].view(BF16)


def shard_inputs(frames, core):
    frames = np.asarray(frames)
    core = np.asarray(core)
    if frames.dtype != np.float32:
        frames = frames.astype(np.float32)
    if core.dtype != np.float32:
        core = core.astype(np.float32)
    if not frames.flags.c_contiguous:
        frames = np.ascontiguousarray(frames)
    if not core.flags.c_contiguous:
        core = np.ascontiguousarray(core)
    fr_bf = _as_bf16_trunc(frames)  # [B,1,C,H,W] bf16 view
    co_bf = _as_bf16_trunc(core)    # [B,1,25,C,H,W] bf16 view
    # One small padded copy per batch (the halo rows / W padding); all
    # per-core entries below are views, so the only large gather happens
    # once, inside the runner (concat / tobytes).
    fp = np.empty((B, C, H + 2 * PAD, WPAD), BF16)
    fp[:, :, :PAD, :] = 0
    fp[:, :, PAD + H:, :] = 0
    fp[:, :, :, :PAD] = 0
    fp[:, :, :, PAD + W:] = 0
    fp[:, :, PAD:PAD + H, PAD:PAD + W] = fr_bf[:, 0]
    in_maps = []
    for k in range(NCORES):
        b, half = k // 2, k % 2
        h0 = half * HH
        in_maps.append({
            "fr": fp[b, :, h0:h0 + HH + 2 * PAD, :],
            "co": co_bf[b, 0, :, :, h0:h0 + HH, :],
        })
    return in_maps


def run(in_maps, **kwargs):
    return run_bass_kernel_spmd(get_nc(), in_maps, list(range(NCORES)), **kwargs)


def kernel(frames, core):
    in_maps = shard_inputs(frames, core)
    res = run(in_maps).results
    outp = np.empty((B, C, H, W), np.float32)
    for k in range(NCORES):
        b, half = k // 2, k % 2
        outp[b, :, half * HH:(half + 1) * HH, :] = res[k]["out"]
    return outp
